# revision 1
# baseline (speedup 1.0000x reference)
"""PriorLSTM Trainium2 kernel (8 NeuronCores, SPMD).

Model: BatchNorm1d(IN) -> 16-layer LSTM(H=128) -> Linear(H->OUT) -> max over T
       -> + prior logits.   B=16, T=32, IN=52686, OUT=2976.

Strategy:
  Phase A (tensor-parallel on IN): each core owns 6656 channels (padded).
    BN folded to per-channel scale/shift; big GEMM gx0.T[g,tok] accumulated
    over 52 K-chunks of 128 channels; AllReduce of the [512,512] partial.
  Phase B (layer pipeline): stage c = layers (2c, 2c+1). States transposed:
    h.T/c.T = [128 h-dim, 16 batch]; weights (pre-transposed on host) are
    the matmul stationary so gates come out as gates.T [128g-block, 16].
    tanh via sigmoid identity (host pre-scales g-gate rows by 2).
    Boundary h crosses cores via one small AllGather per timestep; each
    core selects its predecessor's slice with a 0/1 mask (SPMD-uniform).
    Per-core validity handled by per-tick reset masks from input data.
  Phase C: output projection + temporal max-pool; b_out and prior logits
    folded host-side into one additive constant.

Tokens are time-major: tok = t*16 + b.
"""

import numpy as np

B, T, IN, H, L, OUT = 16, 32, 52686, 128, 16, 2976
EPS = 1e-5
NC = 8
INL = 6656          # channels per core (padded)
NCH = INL // 128    # 52 K-chunks per core
INP = INL * NC      # 53248
TOK = B * T         # 512
OUTP = 3072         # padded OUT
NT = T + NC         # pipeline ticks = 40
NSL = NT + 1        # hist slots


def build_kernel():
    import concourse.bass as bass
    import concourse.bacc as bacc
    import concourse.mybir as mybir
    import concourse.tile as tile

    f32 = mybir.dt.float32
    Alu = mybir.AluOpType
    Act = mybir.ActivationFunctionType

    nc = bacc.Bacc(None, num_devices=NC)

    # ---------------- inputs ------------------------------------------------
    xT = nc.dram_tensor("xT", [INL, TOK], f32, kind="ExternalInput")
    w0T = nc.dram_tensor("w0T", [INL, 512], f32, kind="ExternalInput")
    gam = nc.dram_tensor("gam", [128, NCH], f32, kind="ExternalInput")
    bet = nc.dram_tensor("bet", [128, NCH], f32, kind="ExternalInput")
    wihT = nc.dram_tensor("wihT", [128, 2 * 512], f32, kind="ExternalInput")
    whhT = nc.dram_tensor("whhT", [128, 2 * 512], f32, kind="ExternalInput")
    # btile1[p, gb*16+b] = bias_{layer 2c+1}[gb*128+p]  (broadcast over b)
    btile1 = nc.dram_tensor("btile1", [128, 64], f32, kind="ExternalInput")
    # bcol[p, gb] = bias of the stage's bottom layer (goes into inj)
    bcol = nc.dram_tensor("bcol", [128, 4], f32, kind="ExternalInput")
    m_inj = nc.dram_tensor("m_inj", [128, 1], f32, kind="ExternalInput")
    msel = nc.dram_tensor("msel", [128, NC], f32, kind="ExternalInput")
    mrt = nc.dram_tensor("mrt", [128, NT * 32], f32, kind="ExternalInput")
    woutT = nc.dram_tensor("woutT", [128, OUTP], f32, kind="ExternalInput")
    padd = nc.dram_tensor("padd", [16, OUTP], f32, kind="ExternalInput")

    outp = nc.dram_tensor("outp", [16, OUTP], f32, kind="ExternalOutput")

    with tile.TileContext(nc) as tc:
        with (
            tc.tile_pool(name="big", bufs=1) as big,
            tc.tile_pool(name="wstream", bufs=3) as wst,
            tc.tile_pool(name="small", bufs=2) as small,
            tc.tile_pool(name="ew", bufs=3) as ew,
            tc.tile_pool(name="dram", bufs=4, space="DRAM") as dpool,
        ):
            # ---------------- phase A --------------------------------------
            xtsp_cm = tc.tile_pool(name="xtsp", bufs=1)
            xtsp = xtsp_cm.__enter__()
            SLAB = 13
            xts_l = [xtsp.tile([128, SLAB * TOK], f32, tag=f"xts{s}",
                               name=f"xts{s}") for s in range(4)]

            def xchunk(ch):
                return xts_l[ch // SLAB][
                    :, (ch % SLAB) * TOK:(ch % SLAB + 1) * TOK]
            sums = big.tile([128, NCH], f32, tag="sums")
            sumsq = big.tile([128, NCH], f32, tag="sumsq")
            gams = big.tile([128, NCH], f32, tag="gams")
            bets = big.tile([128, NCH], f32, tag="bets")

            nc.sync.dma_start(out=gams[:], in_=gam[:])
            nc.sync.dma_start(out=bets[:], in_=bet[:])

            xview = xT.rearrange("(c p) t -> p c t", p=128)
            for s in range(4):
                nc.sync.dma_start(
                    out=xts_l[s][:].rearrange("p (c t) -> p c t", t=TOK),
                    in_=xview[:, s * SLAB:(s + 1) * SLAB, :],
                )
            for ch in range(NCH):
                xc = xchunk(ch)
                scr = small.tile([128, TOK], f32, tag="scr")
                nc.vector.tensor_reduce(
                    sums[:, ch:ch + 1], xc, mybir.AxisListType.X, Alu.add,
                )
                nc.vector.scalar_tensor_tensor(
                    out=scr[:], in0=xc, scalar=1.0, in1=xc,
                    op0=Alu.mult, op1=Alu.mult,
                    accum_out=sumsq[:, ch:ch + 1],
                )
            mean = big.tile([128, NCH], f32, tag="mean")
            var = big.tile([128, NCH], f32, tag="var")
            sd = big.tile([128, NCH], f32, tag="sd")
            r0 = big.tile([128, NCH], f32, tag="r0")
            t1 = big.tile([128, NCH], f32, tag="t1")
            scl = big.tile([128, NCH], f32, tag="scl")
            b2 = big.tile([128, NCH], f32, tag="b2")
            nc.vector.tensor_scalar_mul(mean[:], sums[:], 1.0 / TOK)
            nc.vector.tensor_scalar_mul(var[:], sumsq[:], 1.0 / TOK)
            nc.vector.scalar_tensor_tensor(
                out=t1[:], in0=mean[:], scalar=-1.0, in1=mean[:],
                op0=Alu.mult, op1=Alu.mult)
            nc.vector.tensor_tensor(var[:], var[:], t1[:], Alu.add)
            nc.vector.tensor_scalar_add(var[:], var[:], EPS)
            nc.scalar.activation(sd[:], var[:], Act.Sqrt)
            nc.vector.reciprocal(r0[:], sd[:])
            nc.vector.tensor_tensor(t1[:], r0[:], r0[:], Alu.mult)
            nc.vector.tensor_tensor(t1[:], t1[:], var[:], Alu.mult)
            nc.vector.tensor_scalar(
                out=t1[:], in0=t1[:], scalar1=-0.5, scalar2=1.5,
                op0=Alu.mult, op1=Alu.add)
            nc.vector.tensor_tensor(r0[:], r0[:], t1[:], Alu.mult)
            nc.vector.tensor_tensor(scl[:], r0[:], gams[:], Alu.mult)
            nc.vector.scalar_tensor_tensor(
                out=b2[:], in0=mean[:], scalar=-1.0, in1=scl[:],
                op0=Alu.mult, op1=Alu.mult)
            nc.vector.tensor_tensor(b2[:], b2[:], bets[:], Alu.add)

            with tc.tile_pool(name="psgx", bufs=1, space="PSUM") as psgx:
                gx = [psgx.tile([128, TOK], f32, name=f"gx{gb}", tag=f"gx{gb}")
                      for gb in range(4)]
                for ch in range(NCH):
                    xc = xchunk(ch)
                    nc.vector.tensor_scalar(
                        out=xc, in0=xc,
                        scalar1=scl[:, ch:ch + 1], scalar2=b2[:, ch:ch + 1],
                        op0=Alu.mult, op1=Alu.add)
                    wc = wst.tile([128, 512], f32, tag="wc")
                    nc.sync.dma_start(
                        out=wc[:], in_=w0T[ch * 128:(ch + 1) * 128, :])
                    for gb in range(4):
                        nc.tensor.matmul(
                            gx[gb][:], wc[:, gb * 128:(gb + 1) * 128], xc,
                            start=(ch == 0), stop=(ch == NCH - 1),
                            skip_group_check=True)

                gx0 = big.tile([128, 4 * TOK], f32, tag="gx0")
                for gb in range(4):
                    nc.vector.tensor_copy(
                        gx0[:, gb * TOK:(gb + 1) * TOK], gx[gb][:])

            xtsp_cm.__exit__(None, None, None)
            arin = dpool.tile([128, 4 * TOK], f32, tag="arin")
            arout = dpool.tile([128, 4 * TOK], f32, tag="arout")
            nc.gpsimd.dma_start(out=arin[:], in_=gx0[:])
            nc.gpsimd.collective_compute(
                "AllReduce", Alu.add,
                ins=[arin[:].opt()], outs=[arout[:].opt()],
                replica_groups=[list(range(NC))],
            )
            nc.sync.dma_start(out=gx0[:], in_=arout[:])

            # ---------------- phase B --------------------------------------
            wih_s = big.tile([128, 2 * 512], f32, tag="wih")
            whh_s = big.tile([128, 2 * 512], f32, tag="whh")
            bt1_s = big.tile([128, 64], f32, tag="bt1")
            mi_s = small.tile([128, 1], f32, tag="mi")
            bc_s = small.tile([128, 4], f32, tag="bc")
            msel_s = small.tile([128, NC], f32, tag="msel")
            mrt_s = big.tile([128, NT * 32], f32, tag="mrt")
            nc.sync.dma_start(out=wih_s[:], in_=wihT[:])
            nc.sync.dma_start(out=whh_s[:], in_=whhT[:])
            nc.sync.dma_start(out=bt1_s[:], in_=btile1[:])
            nc.sync.dma_start(out=mi_s[:], in_=m_inj[:])
            nc.sync.dma_start(out=bc_s[:], in_=bcol[:])
            nc.sync.dma_start(out=msel_s[:], in_=msel[:])
            nc.sync.dma_start(out=mrt_s[:], in_=mrt[:])
            fence = small.tile([128, 8], f32, tag="fence")
            nc.vector.tensor_copy(fence[:, 0:1], mi_s[:, 0:1])
            nc.vector.tensor_copy(fence[:, 1:2], bc_s[:, 0:1])
            nc.vector.tensor_copy(fence[:, 2:3], msel_s[:, 0:1])
            nc.vector.tensor_copy(fence[:, 3:4], bt1_s[:, 0:1])
            nc.vector.tensor_copy(fence[:, 4:5], mrt_s[:, 0:1])

            # inj[gb, t, b] = m_inj * gx0 + bias_bottom   (padded to NT steps)
            nc.vector.tensor_copy(fence[:, 5:6], gx0[:, 0:1])
            inj = big.tile([128, 4 * NT * 16], f32, tag="inj")
            nc.vector.memset(inj[:], 0.0)
            injv = inj[:, :].rearrange("p (g t b) -> p g t b", g=4, b=16)
            gx0v = gx0[:, :].rearrange("p (g t b) -> p g t b", g=4, b=16)
            for gb in range(4):
                nc.vector.tensor_scalar(
                    out=injv[:, gb:gb + 1, 0:T, :],
                    in0=gx0v[:, gb:gb + 1, :, :],
                    scalar1=mi_s[:, 0:1], scalar2=bc_s[:, gb:gb + 1],
                    op0=Alu.mult, op1=Alu.add)
            for gb in range(4):  # bias-only for the padded tail ticks
                nc.vector.tensor_scalar(
                    out=injv[:, gb:gb + 1, T:NT, :],
                    in0=injv[:, gb:gb + 1, T:NT, :],
                    scalar1=0.0, scalar2=bc_s[:, gb:gb + 1],
                    op0=Alu.mult, op1=Alu.add)

            hist = big.tile([128, NSL * 32], f32, tag="hist")
            cst = big.tile([128, 32], f32, tag="cst")
            nc.vector.memset(hist[:, 0:64], 0.0)
            nc.vector.memset(cst[:], 0.0)

            with tc.tile_pool(name="psb", bufs=3, space="PSUM") as psb:
                for m in range(NT):
                    # --- receive/select predecessor boundary h ---
                    hinc = ew.tile([128, 16], f32, tag="hinc")
                    if m == 0:
                        nc.vector.memset(hinc[:], 0.0)
                    else:
                        agsb = ew.tile([128, NC * 16], f32, tag="agsb")
                        nc.sync.dma_start(
                            out=agsb[:].rearrange(
                                "p (r b) -> p r b", r=NC),
                            in_=ag_out.rearrange("(r p) b -> p r b", p=128))
                        agv = agsb[:, :].rearrange("p (r b) -> p r b", r=NC)
                        nc.vector.tensor_scalar_mul(
                            hinc[:], agv[:, 0:1, :], msel_s[:, 0:1])
                        for r in range(1, NC):
                            nc.vector.scalar_tensor_tensor(
                                out=hinc[:], in0=agv[:, r:r + 1, :],
                                scalar=msel_s[:, r:r + 1], in1=hinc[:],
                                op0=Alu.mult, op1=Alu.add)

                    # --- reset masks (zero garbage state before first use) ---
                    sl = hist[:, m * 32:(m + 1) * 32]
                    mm = mrt_s[:, m * 32:(m + 1) * 32]
                    nc.vector.tensor_tensor(sl, sl, mm, Alu.mult)
                    nc.vector.tensor_tensor(cst[:], cst[:], mm, Alu.mult)

                    # --- gate matmuls (k=0: layer 2c, k=1: layer 2c+1) ---
                    zp = []
                    for k in range(2):
                        p = psb.tile([128, 64], f32, tag=f"gps{k}")
                        zp.append(p)
                        xin = hinc[:] if k == 0 else sl[:, 0:16]
                        hprev = sl[:, k * 16:(k + 1) * 16]
                        for gb in range(4):
                            nc.tensor.matmul(
                                p[:, gb * 16:(gb + 1) * 16],
                                wih_s[:, k * 512 + gb * 128:
                                      k * 512 + (gb + 1) * 128], xin,
                                start=True, stop=False,
                                skip_group_check=True)
                        for gb in range(4):
                            nc.tensor.matmul(
                                p[:, gb * 16:(gb + 1) * 16],
                                whh_s[:, k * 512 + gb * 128:
                                      k * 512 + (gb + 1) * 128], hprev,
                                start=False, stop=True,
                                skip_group_check=True)

                    # --- z = gates + bias/injection, stacked [128,128] ---
                    zs = ew.tile([128, 128], f32, tag="zs")
                    nc.vector.tensor_tensor(
                        zs[:, 0:64], zp[0][:],
                        injv[:, :, m:m + 1, :], Alu.add)
                    nc.vector.tensor_tensor(
                        zs[:, 64:128], zp[1][:], bt1_s[:], Alu.add)

                    sg = ew.tile([128, 128], f32, tag="sg")
                    nc.scalar.activation(sg[:], zs[:], Act.Sigmoid)

                    sgv = sg[:, :].rearrange("p (k g b) -> p k g b", k=2, b=16)
                    i_sl = sgv[:, :, 0:1, :]
                    f_sl = sgv[:, :, 1:2, :]
                    g_sl = sgv[:, :, 2:3, :]
                    o_sl = sgv[:, :, 3:4, :]

                    v = ew.tile([128, 32], f32, tag="v")
                    mt = ew.tile([128, 32], f32, tag="mt")
                    nc.vector.tensor_tensor(cst[:], cst[:], f_sl, Alu.mult)
                    nc.vector.tensor_scalar(
                        out=v[:], in0=g_sl, scalar1=2.0, scalar2=-1.0,
                        op0=Alu.mult, op1=Alu.add)
                    nc.vector.tensor_tensor(mt[:], v[:], i_sl, Alu.mult)
                    nc.vector.tensor_tensor(cst[:], cst[:], mt[:], Alu.add)
                    w2 = ew.tile([128, 32], f32, tag="w2")
                    nc.scalar.activation(w2[:], cst[:], Act.Sigmoid, scale=2.0)
                    # h = sigma(o)*(2*sig(2c)-1) = 2*(sig2c*o) - o
                    h1 = ew.tile([128, 32], f32, tag="h1")
                    nc.vector.tensor_tensor(h1[:], w2[:], o_sl, Alu.mult)
                    nc.vector.scalar_tensor_tensor(
                        out=hist[:, (m + 1) * 32:(m + 2) * 32],
                        in0=h1[:], scalar=2.0, in1=o_sl,
                        op0=Alu.mult, op1=Alu.subtract)

                    # --- ship boundary h (layer 2c+1) via AllGather ---
                    if m < NT - 1:
                        ag_in = dpool.tile([128, 16], f32, tag="agi")
                        ag_out = dpool.tile([128 * NC, 16], f32, tag="ago")
                        nc.sync.dma_start(
                            out=ag_in[:],
                            in_=hist[:, (m + 1) * 32 + 16:(m + 2) * 32])
                        nc.gpsimd.collective_compute(
                            "AllGather", Alu.bypass,
                            ins=[ag_in[:].opt()], outs=[ag_out[:].opt()],
                            replica_groups=[list(range(NC))],
                        )

            # ---------------- phase C --------------------------------------
            pcp_cm = tc.tile_pool(name="pcp", bufs=1)
            pcp = pcp_cm.__enter__()
            wout_s = pcp.tile([128, OUTP], f32, tag="wout", name="wout_s")
            padd_s = pcp.tile([16, OUTP], f32, tag="padd", name="padd_s")
            nc.sync.dma_start(out=wout_s[:], in_=woutT[:])
            nc.sync.dma_start(out=padd_s[:], in_=padd[:])
            msb = pcp.tile([128, OUTP], f32, tag="msb", name="msb")
            histv = hist[:, :].rearrange("p (s x) -> p s x", x=32)
            base = NC + 1  # first slot holding a real top-layer h on stage 7
            with tc.tile_pool(name="psc", bufs=2, space="PSUM") as psc:
                for j in range(4):
                    hstg = pcp.tile([128, 128], f32, tag="hstg",
                                    name=f"hstg{j}", bufs=2)
                    nc.vector.tensor_copy(
                        hstg[:],
                        histv[:, base + 8 * j:base + 8 * j + 8, 16:32])
                    lhs = hstg[:]
                    for ob in range(OUTP // 512):
                        pc = psc.tile([128, 512], f32, tag="pc")
                        nc.tensor.matmul(
                            pc[:], lhs, wout_s[:, ob * 512:(ob + 1) * 512],
                            start=True, stop=True, skip_group_check=True)
                        if j == 0:
                            nc.vector.tensor_copy(
                                msb[:, ob * 512:(ob + 1) * 512], pc[:])
                        else:
                            nc.vector.tensor_tensor(
                                msb[:, ob * 512:(ob + 1) * 512], pc[:],
                                msb[:, ob * 512:(ob + 1) * 512], Alu.max)
            f1 = pcp.tile([64, OUTP], f32, tag="f1", name="f1")
            fs = pcp.tile([64, OUTP], f32, tag="fs", name="fs")
            nc.sync.dma_start(out=fs[:], in_=msb[64:128, :])
            nc.vector.tensor_tensor(f1[:], msb[0:64, :], fs[:], Alu.max)
            nc.sync.dma_start(out=fs[0:32, :], in_=f1[32:64, :])
            nc.vector.tensor_tensor(
                f1[0:32, :], f1[0:32, :], fs[0:32, :], Alu.max)
            nc.sync.dma_start(out=fs[0:16, :], in_=f1[16:32, :])
            nc.vector.tensor_tensor(
                f1[0:16, :], f1[0:16, :], fs[0:16, :], Alu.max)
            nc.vector.tensor_tensor(
                f1[0:16, :], f1[0:16, :], padd_s[:], Alu.add)
            nc.sync.dma_start(out=outp[:], in_=f1[0:16, :])
            pcp_cm.__exit__(None, None, None)

    nc.compile()
    return nc


def prep_inputs(x, bn_gamma, bn_beta, W_ih0, W_ih, W_hh, b_ih, b_hh,
                W_out, b_out, prior):
    """Host-side sharding / layout prep. Returns in_maps list."""
    x = np.asarray(x, np.float32)
    pad = INP - IN
    # time-major tokens: [B,T,IN] -> [T,B,IN] -> [TOK, INP] -> transpose
    xtb = np.ascontiguousarray(x.transpose(1, 0, 2).reshape(TOK, IN))
    xtb = np.pad(xtb, ((0, 0), (0, pad)))
    xT_full = np.ascontiguousarray(xtb.T)             # [INP, TOK]

    W0 = np.array(W_ih0, np.float32)
    W0[2 * H:3 * H, :] *= 2.0
    w0T_full = np.ascontiguousarray(np.pad(W0, ((0, 0), (0, pad))).T)

    gp = np.pad(np.asarray(bn_gamma, np.float32), (0, pad))
    bp = np.pad(np.asarray(bn_beta, np.float32), (0, pad))

    bias = (np.asarray(b_ih, np.float32)
            + np.asarray(b_hh, np.float32)).copy()    # [L, 512]
    bias[:, 2 * H:3 * H] *= 2.0
    Wih = np.array(W_ih, np.float32)                  # [L-1, 512, 128]
    Wih[:, 2 * H:3 * H, :] *= 2.0
    Whh = np.array(W_hh, np.float32)                  # [L, 512, 128]
    Whh[:, 2 * H:3 * H, :] *= 2.0

    WoT = np.zeros((128, OUTP), np.float32)
    WoT[:, :OUT] = np.asarray(W_out, np.float32).T

    p = np.clip(np.asarray(prior, np.float64), 1e-8, 1 - 1e-8)
    logit = (np.log(p) - np.log1p(-p)).astype(np.float32)
    paddv = np.zeros((16, OUTP), np.float32)
    paddv[:, :OUT] = np.asarray(b_out, np.float32)[None, :]
    paddv[:, 1:OUT] += logit[None, :]

    in_maps = []
    for c in range(NC):
        sl = slice(c * INL, (c + 1) * INL)
        l0, l1 = 2 * c, 2 * c + 1
        wihT_c = np.zeros((128, 2 * 512), np.float32)
        if l0 >= 1:
            wihT_c[:, 0:512] = Wih[l0 - 1].T
        wihT_c[:, 512:1024] = Wih[l1 - 1].T
        whhT_c = np.concatenate([Whh[l0].T, Whh[l1].T], axis=1)
        bt1 = np.repeat(bias[l1].reshape(4, 128).T[:, :, None], 16,
                        axis=2).reshape(128, 64)
        bc = np.ascontiguousarray(bias[l0].reshape(4, 128).T)
        mselv = np.zeros((128, NC), np.float32)
        if c > 0:
            mselv[:, c - 1] = 1.0
        mrtv = np.zeros((128, NT, 2, 16), np.float32)
        for m in range(NT):
            mrtv[:, m, 0, :] = 1.0 if m > c else 0.0
            mrtv[:, m, 1, :] = 1.0 if m > c + 1 else 0.0
        in_maps.append({
            "xT": np.ascontiguousarray(xT_full[sl]),
            "w0T": np.ascontiguousarray(w0T_full[sl]),
            "gam": np.ascontiguousarray(gp[sl].reshape(NCH, 128).T),
            "bet": np.ascontiguousarray(bp[sl].reshape(NCH, 128).T),
            "wihT": wihT_c,
            "whhT": np.ascontiguousarray(whhT_c),
            "btile1": bt1,
            "bcol": bc,
            "m_inj": np.full((128, 1), 1.0 if c == 0 else 0.0, np.float32),
            "msel": mselv,
            "mrt": np.ascontiguousarray(mrtv.reshape(128, NT * 32)),
            "woutT": WoT,
            "padd": paddv,
        })
    return in_maps


_CACHED = {}


def kernel(**inputs):
    from concourse.bass_utils import run_bass_kernel_spmd

    if "nc" not in _CACHED:
        _CACHED["nc"] = build_kernel()
    nc = _CACHED["nc"]
    in_maps = prep_inputs(**inputs)
    res = run_bass_kernel_spmd(nc, in_maps, core_ids=list(range(NC)))
    _CACHED["res"] = res
    out = res.results[NC - 1]["outp"][:, :OUT]
    return np.ascontiguousarray(out)


if __name__ == "__main__":
    import reference
    inputs = {k: np.asarray(v) for k, v in reference.setup_inputs().items()}
    got = kernel(**inputs)
    exp = np.asarray(reference.reference(**inputs))
    denom = np.abs(exp).max() + 1e-9
    print("Relative error:", np.abs(got - exp).max() / denom)



# revision 4
# speedup vs baseline: 27742.7219x; 27742.7219x over previous
"""PriorLSTM Trainium2 kernel — zero collectives (8 NeuronCores, SPMD).

Model: BatchNorm1d(IN) -> 16-layer LSTM(H=128) -> Linear(H->OUT) -> max over T
       -> + prior logits.   B=16, T=32, IN=52686, OUT=2976.

Strategy (pure batch data-parallel, NO cross-core communication — measured
~3 ms per collective on this runtime, so the layer-pipelined design was
replaced by per-core replication of the recurrence over 2 batch lanes):
  Each core owns 2 batch lanes (b = 2c, 2c+1) end-to-end.
  Host folds BN (training-mode batch stats) into layer-0: W0' = W_ih0*scl,
  const0 = W_ih0 @ shift + b0; tanh of the g gate computed via 2x-prescaled
  sigmoid (one activation over all 4 gates).  The 16-layer stack strongly
  contracts layer-0 perturbations, so phase A runs in fp8 (e3m4, weights
  pre-scaled x16) and the recurrence in fp16 — verified ~1e-5 output error.
  Phase A (fp8): gx0[512g, 64tok] accumulated over 412 K-chunks of 128
    channels in PSUM; W0' streamed in 26 slabs (DMA-bound, ~27 MB).
  Phase B (fp16 weights/h, f32 c): wavefront over (layer, t): tick m
    processes all layers l with 0 <= m-l < T; per cell 8 tiny matmuls
    (4 gate blocks x ih/hh) into a per-tick PSUM Z tile; biases/gx0
    injected via identity matmul so sigmoid reads PSUM directly; batched
    vector/scalar nonlinearity over all active cells; off-chain ops on the
    Pool engine (NOTE: Pool rejects AluOp max, only mult/add/copy).
  Phase C: output projection + temporal max, overlapped into the wavefront
    on Htop quarters at ticks 22/30/38/46 (pairwise-max tree on DVE);
    b_out + prior logits folded host-side. Host gathers 2 lanes per core.

Token order everywhere: col = t*2 + lane.  Layer rows are stored
"r-major": r = 15 - l, so the active-layer window is contiguous.
"""

import numpy as np

B, T, IN, H, L, OUT = 16, 32, 52686, 128, 16, 2976
EPS = 1e-5
NC = 8
LAN = B // NC            # 2 batch lanes per core
NCH = (IN + 127) // 128  # 412 K-chunks
INP = NCH * 128          # 52736
TOKC = T * LAN           # 64 tokens per core
OUTP = 3072
NOB = OUTP // 128        # 24 output blocks
NT = T + L - 1           # 47 wavefront ticks
SLAB = 16                # w0 chunks per DMA slab
WSCALE = 16.0            # fp8 weight pre-scale (escapes e3m4 denormals)


def build_kernel():
    import concourse.bass as bass
    import concourse.bacc as bacc
    import concourse.mybir as mybir
    import concourse.tile as tile

    f32 = mybir.dt.float32
    f16 = mybir.dt.float16
    fp8 = mybir.dt.float8e3  # e3m4
    Alu = mybir.AluOpType
    Act = mybir.ActivationFunctionType

    nc = bacc.Bacc(None, num_devices=NC)

    xin = nc.dram_tensor("xin", [128, NCH * TOKC], fp8, kind="ExternalInput")
    w0a = nc.dram_tensor("w0a", [128, NCH * 512], fp8, kind="ExternalInput")
    bc0 = nc.dram_tensor("bc0", [128, 4], f32, kind="ExternalInput")
    wih = nc.dram_tensor("wih", [128, L * 512], f16, kind="ExternalInput")
    whh = nc.dram_tensor("whh", [128, L * 512], f16, kind="ExternalInput")
    btl = nc.dram_tensor("btl", [128, L * 4 * LAN], f16, kind="ExternalInput")
    wout = nc.dram_tensor("wout", [128, OUTP], f16, kind="ExternalInput")
    padd = nc.dram_tensor("padd", [128, NOB * LAN], f32, kind="ExternalInput")
    ident = nc.dram_tensor("ident", [128, 128], f16, kind="ExternalInput")

    outp = nc.dram_tensor("outp", [128, NOB * LAN], f32, kind="ExternalOutput")

    with tile.TileContext(nc) as tc:
        with (
            tc.tile_pool(name="big", bufs=1) as big,
            tc.tile_pool(name="wst", bufs=3) as wst,
            tc.tile_pool(name="ew", bufs=3) as ew,
        ):
            xts = big.tile([128, NCH * TOKC], fp8, tag="xts")
            wih_s = big.tile([128, L * 512], f16, tag="wih")
            whh_s = big.tile([128, L * 512], f16, tag="whh")
            btl_s = big.tile([128, L * 4 * LAN], f16, tag="btl")
            bc0_s = big.tile([128, 4], f32, tag="bc0")
            gx0 = big.tile([128, T * 4 * LAN], f16, tag="gx0")
            Hst = big.tile([128, L * LAN], f16, tag="hst")
            Cst = big.tile([128, L * LAN], f32, tag="cst")
            Htop = big.tile([128, TOKC], f16, tag="htop")
            wout_s = big.tile([128, OUTP], f16, tag="wout")
            padd_s = big.tile([128, NOB * LAN], f32, tag="padd")
            outs = big.tile([128, NOB * LAN], f32, tag="outs")
            id_s = big.tile([128, 128], f16, tag="ident")

            # big streams on the sync queue; the rest on the pool queue so
            # real hardware can run them on parallel DMA engines.
            nc.sync.dma_start(out=xts[:], in_=xin[:])
            nc.gpsimd.dma_start(out=wih_s[:], in_=wih[:])
            nc.gpsimd.dma_start(out=whh_s[:], in_=whh[:])
            nc.gpsimd.dma_start(out=btl_s[:], in_=btl[:])
            nc.gpsimd.dma_start(out=bc0_s[:], in_=bc0[:])
            nc.gpsimd.dma_start(out=wout_s[:], in_=wout[:])
            nc.gpsimd.dma_start(out=padd_s[:], in_=padd[:])
            nc.gpsimd.dma_start(out=id_s[:], in_=ident[:])

            nc.vector.memset(Hst[:], 0.0)
            nc.vector.memset(Cst[:], 0.0)

            # ---------------- phase A: gx0 = W0' @ x_c + const0 ------------
            with tc.tile_pool(name="psa", bufs=1, space="PSUM") as psa:
                gxp = psa.tile([128, 4 * TOKC], f32, tag="gxp")
                nslab = (NCH + SLAB - 1) // SLAB
                for s in range(nslab):
                    c0, c1 = s * SLAB, min((s + 1) * SLAB, NCH)
                    wt = wst.tile([128, SLAB * 512], fp8, tag="wt")
                    nc.sync.dma_start(
                        out=wt[:, : (c1 - c0) * 512],
                        in_=w0a[:, c0 * 512:c1 * 512])
                    for c in range(c0, c1):
                        ci = c - c0
                        for gb in range(4):
                            nc.tensor.matmul(
                                gxp[:, gb * TOKC:(gb + 1) * TOKC],
                                wt[:, ci * 512 + gb * 128:
                                   ci * 512 + (gb + 1) * 128],
                                xts[:, c * TOKC:(c + 1) * TOKC],
                                start=(c == 0), stop=(c == NCH - 1),
                                skip_group_check=True)
                # gx0[p, t*8+gb*2+lane] = gxp[p, gb*64+t*2+lane] + bc0[p, gb]
                gx0v = gx0[:, :].rearrange("p (t g l) -> p t g l", g=4, l=LAN)
                gxpv = gxp[:, :].rearrange("p (g t l) -> p g t l", g=4, l=LAN)
                for gb in range(4):
                    nc.vector.tensor_scalar(
                        out=gx0v[:, :, gb:gb + 1, :],
                        in0=gxpv[:, gb:gb + 1, :, :],
                        scalar1=1.0 / WSCALE, scalar2=bc0_s[:, gb:gb + 1],
                        op0=Alu.mult, op1=Alu.add)

            # ---------------- phase B: LSTM wavefront -----------------------
            # (phase C overlapped: output projection runs on Htop quarters
            #  at ticks 22/30/38/46 while the wavefront continues)
            tmpc = big.tile([128, NOB * LAN], f32, tag="tmpc")
            with (
                tc.tile_pool(name="psb", bufs=4, space="PSUM") as psb,
                tc.tile_pool(name="psc", bufs=2, space="PSUM") as psc,
            ):
                for m in range(NT):
                    lmax = min(L - 1, m)
                    lmin = max(0, m - (T - 1))
                    cells = list(range(lmax, lmin - 1, -1))  # descending l
                    n = len(cells)
                    r0 = (L - 1) - lmax
                    Zp = psb.tile([128, 8 * n], f32, tag="zp")

                    # Two half-groups: the next tick's upper-half matmuls can
                    # start as soon as this tick's upper-half h is written.
                    h1 = n  # single group: split variant measured slower
                    groups = []
                    for b0, ge in ((0, h1), (h1, n)):
                        if b0 < ge:
                            groups.append((b0, ge))

                    def emit_mms(b0, ge):
                        gr0 = r0 + b0
                        ng = ge - b0
                        zsl = Zp[:, b0 * 8:ge * 8]
                        # bias/gx0 injection via PE (l=0 btl row is zeros)
                        nc.tensor.matmul(
                            zsl, id_s[:],
                            btl_s[:, gr0 * 8:(gr0 + ng) * 8],
                            start=True, stop=False, skip_group_check=True)
                        if lmin == 0 and ge == n:
                            nc.tensor.matmul(
                                Zp[:, (n - 1) * 8:n * 8], id_s[:],
                                gx0[:, m * 8:(m + 1) * 8],
                                start=False, stop=False,
                                skip_group_check=True)
                        for i in range(b0, ge):
                            l = cells[i]
                            rl = (L - 1) - l
                            rp = rl + 1
                            for gb in range(4):
                                d2 = Zp[:, i * 8 + gb * LAN:
                                        i * 8 + (gb + 1) * LAN]
                                wsl = slice((rl * 4 + gb) * 128,
                                            (rl * 4 + gb + 1) * 128)
                                if l >= 1:
                                    nc.tensor.matmul(
                                        d2, wih_s[:, wsl],
                                        Hst[:, rp * LAN:(rp + 1) * LAN],
                                        start=False, stop=False,
                                        skip_group_check=True)
                                nc.tensor.matmul(
                                    d2, whh_s[:, wsl],
                                    Hst[:, rl * LAN:(rl + 1) * LAN],
                                    start=False, stop=True,
                                    skip_group_check=True)

                    sgs = []
                    for gi, (b0, ge) in enumerate(groups):
                        emit_mms(b0, ge)
                        sg = ew.tile([128, 8 * (ge - b0)], f32,
                                     tag=f"sg{gi}")
                        nc.scalar.activation(
                            sg[:, :], Zp[:, b0 * 8:ge * 8], Act.Sigmoid)
                        sgs.append(sg)

                    for gi, (b0, ge) in enumerate(groups):
                        ng = ge - b0
                        gr0 = r0 + b0
                        sg = sgs[gi]
                        sgv = sg[:, :].rearrange(
                            "p (c g l) -> p c g l", g=4, l=LAN)
                        i_sl = sgv[:, :, 0:1, :]
                        f_sl = sgv[:, :, 1:2, :]
                        g_sl = sgv[:, :, 2:3, :]
                        o_sl = sgv[:, :, 3:4, :]
                        cs = Cst[:, gr0 * LAN:(gr0 + ng) * LAN]
                        hs = Hst[:, gr0 * LAN:(gr0 + ng) * LAN]
                        v = ew.tile([128, LAN * ng], f32, tag=f"v{gi}")
                        th = ew.tile([128, LAN * ng], f32, tag=f"th{gi}")
                        # v-ops feed cs+v only; run them on the idle Pool
                        # engine in parallel with cs*f on DVE
                        nc.gpsimd.tensor_scalar(
                            out=v[:], in0=g_sl, scalar1=2.0, scalar2=-1.0,
                            op0=Alu.mult, op1=Alu.add)
                        nc.gpsimd.tensor_tensor(v[:], v[:], i_sl, Alu.mult)
                        nc.vector.tensor_tensor(cs, cs, f_sl, Alu.mult)
                        nc.vector.tensor_tensor(cs, cs, v[:], Alu.add)
                        nc.scalar.activation(th[:], cs, Act.Tanh)
                        nc.vector.tensor_tensor(hs, th[:], o_sl, Alu.mult)
                        if gi == 0 and m >= L - 1:
                            # f32 top-layer h for the output head
                            t15 = m - (L - 1)
                            nc.gpsimd.tensor_tensor(
                                Htop[:, t15 * LAN:(t15 + 1) * LAN],
                                th[:, 0:LAN], sgv[:, 0:1, 3:4, :], Alu.mult)

                    if m >= 22 and (m - 22) % 8 == 0:
                        q = (m - 22) // 8
                        dst = outs if q == 0 else tmpc
                        pcq = psc.tile([128, NOB * 16], f32, tag="pcq")
                        for ob in range(NOB):
                            nc.tensor.matmul(
                                pcq[:, ob * 16:(ob + 1) * 16],
                                wout_s[:, ob * 128:(ob + 1) * 128],
                                Htop[:, q * 16:(q + 1) * 16],
                                start=True, stop=True, skip_group_check=True)
                        st0 = ew.tile([128, NOB * 16], f32, tag="st0")
                        st1 = ew.tile([128, NOB * 8], f32, tag="st1")
                        nc.vector.tensor_copy(st0[:], pcq[:])
                        s0v = st0[:, :].rearrange(
                            "p (o t l) -> p o t l", t=8, l=LAN)
                        s1v = st1[:, :].rearrange(
                            "p (o t l) -> p o t l", t=4, l=LAN)
                        nc.vector.tensor_tensor(
                            st1[:], s0v[:, :, 0:4, :], s0v[:, :, 4:8, :],
                            Alu.max)
                        nc.vector.tensor_tensor(
                            st0[:, :NOB * 4], s1v[:, :, 0:2, :],
                            s1v[:, :, 2:4, :], Alu.max)
                        s2v = st0[:, :NOB * 4].rearrange(
                            "p (o t l) -> p o t l", t=2, l=LAN)
                        nc.vector.tensor_tensor(
                            dst[:, :], s2v[:, :, 0:1, :], s2v[:, :, 1:2, :],
                            Alu.max)
                        if q > 0:
                            nc.vector.tensor_tensor(
                                outs[:], outs[:], tmpc[:], Alu.max)

            nc.gpsimd.tensor_tensor(outs[:], outs[:], padd_s[:], Alu.add)
            nc.sync.dma_start(out=outp[:], in_=outs[:])

    nc.compile()
    return nc


def prep_inputs(x, bn_gamma, bn_beta, W_ih0, W_ih, W_hh, b_ih, b_hh,
                W_out, b_out, prior):
    """Host-side: BN fold, layouts, sharding. Returns in_maps list."""
    import ml_dtypes
    fp8 = ml_dtypes.float8_e3m4

    x = np.asarray(x, np.float32)
    xm = x.reshape(B * T, IN)
    mean = xm.mean(0, dtype=np.float64)
    var = xm.var(0, dtype=np.float64)
    scl = (np.asarray(bn_gamma, np.float64) / np.sqrt(var + EPS)).astype(
        np.float32)
    shift = (np.asarray(bn_beta, np.float32) - mean.astype(np.float32) * scl)

    W0 = np.asarray(W_ih0, np.float32)
    const0 = (W0 @ shift + np.asarray(b_ih, np.float32)[0]
              + np.asarray(b_hh, np.float32)[0])
    W0p = W0 * scl[None, :]
    W0p[2 * H:3 * H] *= 2.0
    const0[2 * H:3 * H] *= 2.0

    w0aT = np.zeros((INP, 512), np.float32)
    w0aT[:IN] = W0p.T * WSCALE
    w0a = np.ascontiguousarray(
        w0aT.reshape(NCH, 128, 512).transpose(1, 0, 2)
    ).reshape(128, NCH * 512).astype(fp8)

    bc0 = np.ascontiguousarray(const0.reshape(4, 128).T)

    Wih = np.asarray(W_ih, np.float32)   # [L-1, 512, 128]
    Whh = np.asarray(W_hh, np.float32)   # [L,   512, 128]
    bias = (np.asarray(b_ih, np.float32) + np.asarray(b_hh, np.float32)).copy()
    Wih = Wih.copy()
    Whh = Whh.copy()
    Wih[:, 2 * H:3 * H, :] *= 2.0
    Whh[:, 2 * H:3 * H, :] *= 2.0
    bias[:, 2 * H:3 * H] *= 2.0

    wihT = np.zeros((128, L, 512), np.float32)
    whhT = np.zeros((128, L, 512), np.float32)
    btl = np.zeros((128, L, 4, LAN), np.float32)
    for l in range(L):
        r = (L - 1) - l
        if l >= 1:
            wihT[:, r, :] = Wih[l - 1].T
            btl[:, r, :, :] = bias[l].reshape(4, 128).T[:, :, None]
        whhT[:, r, :] = Whh[l].T
    wihT = np.ascontiguousarray(wihT.reshape(128, L * 512)).astype(np.float16)
    whhT = np.ascontiguousarray(whhT.reshape(128, L * 512)).astype(np.float16)
    btl = np.ascontiguousarray(btl.reshape(128, L * 4 * LAN)).astype(np.float16)

    woutT = np.zeros((128, OUTP), np.float16)
    woutT[:, :OUT] = np.asarray(W_out, np.float32).T.astype(np.float16)

    p64 = np.clip(np.asarray(prior, np.float64), 1e-8, 1 - 1e-8)
    logit = (np.log(p64) - np.log1p(-p64)).astype(np.float32)
    addv = np.zeros((OUTP,), np.float32)
    addv[:OUT] = np.asarray(b_out, np.float32)
    addv[1:OUT] += logit
    paddv = np.ascontiguousarray(
        np.repeat(addv.reshape(NOB, 128).T[:, :, None], LAN, axis=2)
    ).reshape(128, NOB * LAN)

    in_maps = []
    for c in range(NC):
        xs = x[LAN * c:LAN * (c + 1)]          # [2, 32, IN]
        xt = np.zeros((INP, T, LAN), np.float32)
        xt[:IN] = xs.transpose(2, 1, 0)        # [IN, T, LAN]
        xin_c = np.ascontiguousarray(
            xt.reshape(NCH, 128, TOKC).transpose(1, 0, 2)
        ).reshape(128, NCH * TOKC).astype(fp8)
        in_maps.append({
            "xin": xin_c,
            "w0a": w0a,
            "bc0": bc0,
            "wih": wihT,
            "whh": whhT,
            "btl": btl,
            "wout": woutT,
            "padd": paddv,
            "ident": np.eye(128, dtype=np.float16),
        })
    return in_maps


def gather_out(results):
    """results: list of per-core dicts with 'outp' [128, NOB*LAN]."""
    out = np.zeros((B, OUT), np.float32)
    for c in range(NC):
        op = np.asarray(results[c]["outp"], np.float32).reshape(128, NOB, LAN)
        for lane in range(LAN):
            flat = np.ascontiguousarray(op[:, :, lane].T).reshape(OUTP)
            out[LAN * c + lane] = flat[:OUT]
    return out


_CACHED = {}


def kernel(**inputs):
    from concourse.bass_utils import run_bass_kernel_spmd

    if "nc" not in _CACHED:
        _CACHED["nc"] = build_kernel()
    nc = _CACHED["nc"]
    in_maps = prep_inputs(**inputs)
    res = run_bass_kernel_spmd(nc, in_maps, core_ids=list(range(NC)))
    return gather_out(res.results)


if __name__ == "__main__":
    import reference
    inputs = {k: np.asarray(v) for k, v in reference.setup_inputs().items()}
    got = kernel(**inputs)
    exp = np.asarray(reference.reference(**inputs))
    denom = np.abs(exp).max() + 1e-9
    print("Relative error:", np.abs(got - exp).max() / denom)
